# revision 14
# baseline (speedup 1.0000x reference)
"""Distributed Trainium2 kernel for the audio-visual contrastive loss.

Math (reference):
    a = l2norm(audio)  (B=32, Na=512, D=768)
    v = l2norm(visual) (B=32, Nv=256, D=768)
    token_sims[b,c,n,m] = (a[b,n] . v[c,m]) / T
    clip_sims = mean_n max_m token_sims          (B, B)
    loss = mean_b -0.5*(log_softmax(clip)[b,b] + log_softmax(clip.T)[b,b])

Distribution over 8 NeuronCores (same collective plan as the proven
baseline): audio/visual batch sharded 4 clips/core, visual normalized,
fp8-cast, transposed d-major and AllGather'd in 4 chunks that pipeline
with the main loop; ring-relative consumption; scalar-loss AllReduce.

Main-loop engine plan (v3):
  - PE: k2-outer supergroups.  Each supergroup = one audio row-tile x
    the vTf blocks made ready by the latest gather chunk; for each of
    the 3 DoubleRow contraction chunks one LDWEIGHTS feeds every block
    in the supergroup (a post-legalize pass drops the redundant
    LDWEIGHTS the legalizer inserts per matmul -- the PE array keeps
    its weights), cutting PE weight-load time ~4x.
  - PSUM: 6 independent single-bank tiles cycled round-robin; each
    bank drains right after its k2=2 matmul, so reuse costs no bubble.
  - bank drain: ACT (idle in the main loop) copies the bank to SBUF
    as bf16; DVE does a 2x-mode tensor_max halving then a short
    max-reduce into mx.  Wave-0 banks (before ACT is free) reduce
    directly on DVE.  GPSIMD cannot touch PSUM and cannot run
    tensor_tensor, so it instead handles SBUF-only work: the
    own-clip vTf copies and the visual/audio-batch-2 row
    normalization (normalize_recip ucode), off-loading ACT.
  - prep transposes run k-outer through one shared PSUM bank (8
    rotating 256B sub-slots) and are drained by 4-tile-wide DVE
    copies instead of 1-tile copies.
"""

import sys

for _p in ("/opt/trn_rl_repo",):
    if _p not in sys.path:
        sys.path.insert(0, _p)

import numpy as np

import concourse.bacc as bacc
import concourse.bass as bass_mod
import concourse.mybir as mybir
import concourse.tile as tile
import concourse.tile_legalize as tl_mod
from concourse.tile_rust import add_dep_helper

N_CORES = 8
B = 32
NA = 512
NV = 256
D = 768
TEMPERATURE = 0.1
BL = B // N_CORES            # 4 clips per core
AROWS = BL * NA              # 2048 audio rows per core
VROWS = BL * NV              # 1024 visual rows per core
KD = D // 128                # 6 contraction chunks
KD2 = KD // 2                # 3 DoubleRow chunk-pairs
NT_A = AROWS // 128          # 16 audio row-tiles
NT_V = VROWS // 128          # 8 visual row-tiles
G = 4                        # visual AllGather chunks (1 clip/core each)
VCH = VROWS // G             # 256 visual rows per chunk per core
NBANK = 6                    # PSUM banks used by the main-loop ring
USE_NR = True                # GPSIMD normalize_recip for vis + audio batch 2
F32 = mybir.dt.float32
BF16 = mybir.dt.bfloat16
FP8 = mybir.dt.float8e4
AX = mybir.AxisListType
ALU = mybir.AluOpType
ACT = mybir.ActivationFunctionType
SCL = 16.0                   # fp8 pre-scale (folded into the norm rsqrt)
SC = 1.0 / (NA * TEMPERATURE * SCL * SCL)   # psum-count -> clip_sims scale

# ---------------------------------------------------------------------------
# Post-legalize LDWEIGHTS dedupe.  tile_legalize splits every InstMatmult
# into InstLdweights + InstMatmult even when consecutive matmuls use
# identical stationary weights; a DoubleRow LDWEIGHTS costs ~156ns vs the
# ~107ns matmul stream, and the PE array keeps its weights between
# matmuls, so redundant reloads are pure loss.  Walk each block in final
# emission order tracking the loaded-weights signature and drop repeats,
# rewiring dep edges onto the matmul the dropped LDWEIGHTS was split from
# (which follows it in the same PE FIFO).
# ---------------------------------------------------------------------------

_ORIG_LEGALIZE = tl_mod.tile_legalize


def _wsig(ldw):
    ap = ldw.ins[0]
    bap = getattr(ap, "bass_ap", None)
    if bap is not None:
        key = (getattr(bap.tensor, "name", str(bap.tensor)), bap.offset,
               str(bap.ap))
    else:
        key = (getattr(ap, "memref", None), ap.offset, str(ap.ap))
    return (key, str(ap.dtype), str(ldw.perf_mode), bool(ldw.is_transpose),
            ldw.tile_position, ldw.tile_size)


def _dedup_tile_legalize(ordered, nc):
    out = _ORIG_LEGALIZE(ordered, nc)
    removed = []
    for bb, insts in out.items():
        cur = None
        keep = []
        n = len(insts)
        i = 0
        while i < n:
            inst = insts[i]
            if isinstance(inst, mybir.InstLdweights):
                sig = _wsig(inst)
                if cur is not None and sig == cur:
                    mm = None
                    for j in range(i + 1, n):
                        nxt = insts[j]
                        if isinstance(nxt, mybir.InstMatmult):
                            mm = nxt
                            break
                        if isinstance(nxt, mybir.InstLdweights):
                            break
                    if mm is not None:
                        removed.append((inst, mm))
                        i += 1
                        continue
                cur = sig
            keep.append(inst)
            i += 1
        out[bb] = keep
    if removed:
        rm_id = {id(l): m for l, m in removed}
        rm_nm = {l.name: m for l, m in removed}
        for bb, insts in out.items():
            for inst in insts:
                desc = getattr(inst, "descendants", None)
                if not desc:
                    continue
                new = []
                changed = False
                for d in desc:
                    m = rm_id.get(id(d)) or rm_nm.get(getattr(d, "name", None))
                    if m is not None:
                        new.append(m)
                        changed = True
                    else:
                        new.append(d)
                if changed:
                    inst.descendants = new
        for ldw, mm in removed:
            deps = getattr(ldw, "dependencies", None)
            if deps:
                cur = list(getattr(mm, "dependencies", None) or [])
                cur.extend(deps)
                mm.dependencies = cur
        imap = getattr(nc, "inst_map", None)
        if isinstance(imap, dict):
            for ldw, _ in removed:
                imap.pop(ldw.name, None)
    return out


def _install_dedup():
    if tl_mod.tile_legalize is not _dedup_tile_legalize:
        tl_mod.tile_legalize = _dedup_tile_legalize
    if tile.tile_legalize is not _dedup_tile_legalize:
        tile.tile_legalize = _dedup_tile_legalize


def pos_of_clip(c, core):
    # ring-relative positions: clip c = 4*r + g sits at q = g*8 + t where
    # t = (r - core) mod 8 is the ring step (t=0 is the core's own clip).
    g = c % G
    r = c // BL
    t = (r - core) % N_CORES
    return g * 8 + t


def build():
    _install_dedup()
    nc = bacc.Bacc("TRN2", target_bir_lowering=False, debug=False,
                   num_devices=N_CORES)
    a_in = nc.declare_dram_parameter("audio", [AROWS, D], F32, isOutput=False)
    v_in = nc.declare_dram_parameter("visual", [VROWS, D], F32,
                                     isOutput=False)
    dmask_in = nc.declare_dram_parameter("dmask", [1, 128], F32,
                                         isOutput=False)
    perm_in = nc.declare_dram_parameter("perm", [32, 32], F32,
                                        isOutput=False)
    out = nc.declare_dram_parameter("out", [1, 1], F32, isOutput=True)
    ident_dram = nc.inline_tensor(np.eye(128, dtype=np.float32), name="ident")
    rg = [list(range(N_CORES))]

    with tile.TileContext(nc) as tc:
        with (
            tc.tile_pool(name="persist", bufs=1) as pp,
            tc.tile_pool(name="work", bufs=3) as wp,
            tc.tile_pool(name="ps", bufs=2, space="PSUM") as ps,
            tc.tile_pool(name="dram", bufs=1, space="DRAM") as dp,
        ):
            # ---- constants ------------------------------------------------
            ident_f32 = pp.tile([128, 128], F32, tag="identf")
            nc.sync.dma_start(out=ident_f32[:], in_=ident_dram[:])
            ident_bf = pp.tile([128, 128], BF16, tag="identb")
            nc.scalar.copy(ident_bf[:], ident_f32[:])
            ones = pp.tile([128, 1], F32, tag="ones")
            nc.gpsimd.memset(ones[:], 1.0)
            # warm the ACT Ln/Exp tables off the critical path
            wrma = wp.tile([1, 1], F32, tag="wrma")
            nc.vector.memset(wrma[:], 1.0)
            wrmb = wp.tile([1, 1], F32, tag="wrmb")
            nc.scalar.activation(wrmb[:], wrma[:], ACT.Exp)
            wrmc = wp.tile([1, 1], F32, tag="wrmc")
            nc.scalar.activation(wrmc[:], wrmb[:], ACT.Ln)
            vec = pp.tile([1, 64], F32, tag="vec")
            nc.vector.memset(vec[:], 0.0)

            # ---- persistent tensors ---------------------------------------
            VTW = N_CORES * VROWS        # 8192 vT columns per d-chunk
            aTf = [pp.tile([128, 2 * AROWS], FP8, tag=f"aT8{k2}",
                           name=f"aT8{k2}") for k2 in range(KD2)]
            vstall = pp.tile([128, KD * VROWS], FP8, tag="vstall")
            vTf = [pp.tile([128, 2 * VTW], FP8, tag=f"vT8{k2}",
                           name=f"vT8{k2}") for k2 in range(KD2)]
            mx = pp.tile([128, 512], F32, tag="mx")

            # ---- row-tile prep --------------------------------------------
            # one PSUM bank shared by all transposes: 8 rotating 256B
            # sub-slots (range-tracked deps).  Transposes run k-outer so
            # 4 consecutive tiles land in adjacent sub-slots and drain
            # with one wide DVE copy.
            ptbig = ps.tile([128, 1024], BF16, tag="pt", name="ptbig",
                            bufs=1)
            tr_ctr = [0]
            prev_pool = [None]

            def pool_chain(inst):
                if prev_pool[0] is not None:
                    add_dep_helper(inst.ins, prev_pool[0].ins, sync=False,
                                   reason="pool chain")
                prev_pool[0] = inst

            class BatchRec:
                pass

            def prep_batch(src, t0, nb, dst_of_k, load_group, use_nr):
                rec = BatchRec()
                raws = []
                ssb = wp.tile([128, nb], F32, tag="ssb", name="ssb", bufs=2)
                for j in range(nb):
                    t = t0 + j
                    raw = wp.tile([128, D], F32, tag="raw", name="raw",
                                  bufs=10)
                    load_group.append(
                        nc.sync.dma_start(out=raw[:],
                                          in_=src[t * 128:(t + 1) * 128, :]))
                    sq = nc.scalar.activation(
                        wp.tile([128, D], F32, tag="sqs", name="sqs",
                                bufs=2)[:],
                        raw[:], ACT.Square, accum_out=ssb[:, j:j + 1])
                    if j == 0:
                        rec.sq_first = sq
                    raws.append(raw)
                nrm = wp.tile([128, nb], F32, tag="nrm", name="nrm", bufs=2)
                rec.cast_last = nc.scalar.activation(
                    nrm[:], ssb[:], ACT.Sqrt, scale=1.0 / (SCL * SCL))
                nbfs = []
                if use_nr:
                    # GPSIMD divides by the norm (and writes bf16) so the
                    # ACT engine stays free for main-loop bank movers
                    for j in range(nb):
                        nbf = wp.tile([128, D], BF16, tag="nbf", name="nbf",
                                      bufs=10)
                        nr = nc.gpsimd.normalize_recip(
                            nbf[:], raws[j][:], nrm[:, j:j + 1])
                        pool_chain(nr)
                        nbfs.append(nbf)
                else:
                    rnb = wp.tile([128, nb], F32, tag="rnb", name="rnb",
                                  bufs=2)
                    nc.vector.reciprocal(rnb[:], nrm[:])
                    for j in range(nb):
                        nbf = wp.tile([128, D], BF16, tag="nbf", name="nbf",
                                      bufs=10)
                        rec.cast_last = nc.scalar.activation(
                            nbf[:], raws[j][:], ACT.Copy, bias=0.0,
                            scale=rnb[:, j:j + 1])
                        nbfs.append(nbf)
                rec.tr_first = rec.tr_last = None
                rec.cp_first = rec.cp_last = None
                for k in range(KD):
                    dst_tile, col = dst_of_k(t0, k)
                    c0 = 0
                    slot0 = tr_ctr[0] % 8
                    for j in range(nb):
                        slot = tr_ctr[0] % 8
                        tr_ctr[0] += 1
                        tr = nc.tensor.transpose(
                            ptbig[:, slot * 128:(slot + 1) * 128],
                            nbfs[j][:, 128 * k:128 * (k + 1)],
                            ident_bf[:])
                        if rec.tr_first is None:
                            rec.tr_first = tr
                        rec.tr_last = tr
                        if j + 1 == nb or (j + 1) % 4 == 0:
                            cw = j + 1 - c0
                            cp = nc.vector.tensor_copy(
                                dst_tile[:, col + c0 * 128:
                                         col + (j + 1) * 128],
                                ptbig[:, slot0 * 128:
                                      (slot0 + cw) * 128])
                            if rec.cp_first is None:
                                rec.cp_first = cp
                            rec.cp_last = cp
                            c0 = j + 1
                            slot0 = tr_ctr[0] % 8
                return rec

            # ---- visual prep + bounce + chunked AllGather -----------------
            vis_loads, aud_loads1, aud_loads2 = [], [], []
            vt_loads = []
            vgath = []
            ag_insts = []
            vis_recs, aud_recs = [], []
            nbv = NT_V // G              # 2 tiles per chunk
            vst3 = vstall[:].rearrange("p (k c) -> p k c", k=KD)
            for g in range(G):
                vis_recs.append(prep_batch(
                    v_in, g * nbv, nbv,
                    lambda t0, k: (vstall, k * VROWS + (t0 // nbv) * VCH),
                    vis_loads, USE_NR))
                vb = dp.tile([128, KD * VCH // 4], F32, tag=f"vb{g}",
                             name=f"vb{g}")
                nc.scalar.dma_start(
                    out=vb[:, :].rearrange("p (k c) -> p k c", k=KD),
                    in_=vst3[:, :, g * VCH:(g + 1) * VCH].bitcast(F32))
                vg = dp.tile([N_CORES * 128, KD * VCH // 4], F32,
                             tag=f"vg{g}", name=f"vg{g}",
                             addr_space="Shared")
                cc = nc.gpsimd.collective_compute(
                    "AllGather", ALU.bypass, replica_groups=rg,
                    ins=[vb[:, :].opt()], outs=[vg[:, :].opt()])
                ag_insts.append(cc)
                vgath.append(vg)

            dmask = pp.tile([1, 128], F32, tag="dmask")
            vis_loads.append(nc.sync.dma_start(out=dmask[:], in_=dmask_in[:]))
            perm = pp.tile([32, 32], F32, tag="perm")
            vis_loads.append(nc.sync.dma_start(out=perm[:], in_=perm_in[:]))

            # ---- own clips: vstall -> vTf local blocks (Pool tensor_copy,
            # SBUF->SBUF; ordered right after the visual normalize work) ---
            loc_last = None
            for pair in range(2):
                fL = 14 + pair
                for k2 in range(KD2):
                    dst = vTf[k2][:, fL * 1024:(fL + 1) * 1024].bitcast(
                        F32).rearrange("p (ko h n) -> p ko h n", ko=2, h=2)
                    src = vst3[:, 2 * k2:2 * k2 + 2,
                               2 * pair * VCH:
                               (2 * pair + 2) * VCH].bitcast(F32).rearrange(
                        "p ko (h n) -> p ko h n", h=2)
                    cp = nc.gpsimd.tensor_copy(dst, src)
                    pool_chain(cp)
                    loc_last = cp

            # ---- audio prep (batch 1 via ACT casts for the fastest
            # lead-in; batch 2 via GPSIMD normalize_recip) ------------------
            aud_recs.append(prep_batch(
                a_in, 0, 8,
                lambda t0_, k: (aTf[k // 2], (k % 2) * AROWS + t0_ * 128),
                aud_loads1, False))
            aud_recs.append(prep_batch(
                a_in, 8, 8,
                lambda t0_, k: (aTf[k // 2], (k % 2) * AROWS + t0_ * 128),
                aud_loads2, USE_NR))

            # ---- scheduler pins: visual prep, then audio prep, per engine
            add_dep_helper(aud_recs[0].sq_first.ins,
                           vis_recs[-1].cast_last.ins, sync=False,
                           reason="act: visual prep first")
            add_dep_helper(aud_recs[0].tr_first.ins,
                           vis_recs[-1].tr_last.ins, sync=False,
                           reason="pe: visual prep first")
            add_dep_helper(aud_recs[0].cp_first.ins,
                           vis_recs[-1].cp_last.ins, sync=False,
                           reason="dve: audio after visual")

            # ---- main loop ------------------------------------------------
            banks = [ps.tile([128, 512], F32, tag=f"bk{i}", name=f"bk{i}",
                             bufs=1) for i in range(NBANK)]
            bank_reader = [None] * NBANK
            wptr = [0]
            first_mm = [True]
            prev_mm = [None]
            prev_dve = [aud_recs[0].cp_last]
            prev_act = [aud_recs[-1].cast_last]
            mx4 = mx[:].rearrange("p (nt b q) -> p nt b q", nt=NA // 128,
                                  b=BL)

            def out_ap_for(f, b, nt):
                mxq = mx4[:, nt, b, :]
                if f >= 14:
                    p_ = f - 14
                    return mxq.rearrange("p (g q) -> p g q",
                                         g=G)[:, 2 * p_:2 * p_ + 2, 0:1]
                if f >= 12:
                    p_ = f - 12
                    return mxq.rearrange("p (g q) -> p g q",
                                         g=G)[:, 2 * p_:2 * p_ + 2, 7:8]
                g_, u = divmod(f, 3)
                q0 = g_ * 8 + 2 * u + 1
                return mxq[:, q0:q0 + 2]

            def chain(prev_ref, inst, reason):
                if prev_ref[0] is not None:
                    add_dep_helper(inst.ins, prev_ref[0].ins, sync=False,
                                   reason=reason)
                prev_ref[0] = inst

            def emit_reduce(bk_i, f, b, nt, lane):
                bk = banks[bk_i]
                oap = out_ap_for(f, b, nt)
                if lane == "act":
                    mv = wp.tile([128, 512], BF16, tag="mv", name="mv",
                                 bufs=8)
                    mvr = nc.scalar.copy(mv[:], bk[:])
                    chain(prev_act, mvr, "act chain")
                    mv4 = mv[:].rearrange("p (c h m) -> p c h m", c=2, h=2)
                    scr = wp.tile([128, 256], BF16, tag="scr", name="scr",
                                  bufs=8)
                    hv = nc.vector.tensor_max(
                        out=scr[:].rearrange("p (c m) -> p c m", c=2),
                        in0=mv4[:, :, 0, :], in1=mv4[:, :, 1, :])
                    chain(prev_dve, hv, "dve chain")
                    rd = nc.vector.tensor_reduce(
                        out=oap,
                        in_=scr[:].rearrange("p (c m) -> p c m", c=2),
                        axis=AX.X, op=ALU.max)
                    chain(prev_dve, rd, "dve chain")
                    bank_reader[bk_i] = mvr
                else:
                    rd = nc.vector.tensor_reduce(
                        out=oap,
                        in_=bk[:].rearrange("p (c m) -> p c m", c=2),
                        axis=AX.X, op=ALU.max)
                    chain(prev_dve, rd, "dve chain")
                    bank_reader[bk_i] = rd

            def sg(b, nt, fs, lane):
                lcol = (b * (NA // 128) + nt) * 128
                bks = []
                for _ in fs:
                    bks.append(wptr[0] % NBANK)
                    wptr[0] += 1
                for k2 in range(KD2):
                    lhs3 = aTf[k2][:].rearrange(
                        "p (ko m) -> p ko m", ko=2)[:, :, lcol:lcol + 128]
                    for s, f in enumerate(fs):
                        rhs3 = vTf[k2][:].rearrange(
                            "p (f ko n) -> p f ko n", ko=2, n=512)[:, f]
                        mm = nc.tensor.matmul(
                            banks[bks[s]][:], lhsT=lhs3, rhs=rhs3,
                            start=(k2 == 0), stop=(k2 == KD2 - 1),
                            perf_mode=mybir.MatmulPerfMode.DoubleRow)
                        if first_mm[0]:
                            add_dep_helper(mm.ins, aud_recs[0].tr_last.ins,
                                           sync=False,
                                           reason="pe prep before main")
                            first_mm[0] = False
                        chain(prev_mm, mm, "pe chain")
                        if k2 == 0 and bank_reader[bks[s]] is not None:
                            add_dep_helper(mm.ins,
                                           bank_reader[bks[s]].ins,
                                           sync=True, reason="bank WAR")
                for s, f in enumerate(fs):
                    emit_reduce(bks[s], f, b, nt, lane)

            # ring loads for gather chunk g -> vTf (emitted per wave)
            last_load = {}
            pid = nc.sync.partition_id()
            pid_act = nc.scalar.partition_id()
            ROWBLK = 128 * (KD * VCH // 4)    # f32 elements per rank block

            def emit_ring_loads(g):
                for t in range(1, N_CORES):
                    if t == 7:
                        f, half = 12 + g // 2, g % 2
                    else:
                        f, half = 3 * g + (t - 1) // 2, (t - 1) % 2
                    r = (pid + t) % N_CORES
                    r_act = (pid_act + t) % N_CORES
                    blk0 = vgath[g][0:128, :].rearrange(
                        "p (k c) -> p k c", k=KD)
                    for k2 in range(KD2):
                        s_ap = blk0[:, 2 * k2:2 * k2 + 2, :]
                        roff = r if k2 < 2 else r_act
                        dyn = bass_mod.AP(
                            tensor=s_ap.tensor,
                            offset=roff * ROWBLK + s_ap.offset,
                            ap=s_ap.ap,
                            dep_tracking_offset=s_ap.offset)
                        dst = vTf[k2][:, f * 1024:(f + 1) * 1024].bitcast(
                            F32).rearrange("p (ko n) -> p ko n", ko=2)
                        if k2 < 2:
                            h1 = nc.sync.dma_start(
                                out=dst[:, :, half * 64:half * 64 + 64],
                                in_=dyn)
                            vt_loads.append(h1)
                            last_load[(g, 0)] = h1
                        else:
                            h2 = nc.scalar.dma_start(
                                out=dst[:, :, half * 64:half * 64 + 64],
                                in_=dyn)
                            chain(prev_act, h2, "act chain")
                            last_load[(g, 1)] = h2
                if g < G - 1:
                    for ring in range(2):
                        h = last_load.get((g, ring))
                        if h is not None:
                            add_dep_helper(ag_insts[g + 1].ins, h.ins,
                                           sync=True,
                                           reason="AG waits prev chunk loads")

            # wave 0: own clips (blocks 14,15) for all 16 audio tiles,
            # DVE-direct reduces (ACT is still busy with audio prep here)
            emit_ring_loads(0)
            for b in range(BL):
                for nt in range(NA // 128):
                    sg(b, nt, [14, 15], "dve")
            # chunk waves in gather-arrival order; step-7 blocks join the
            # wave whose gather completes them (12 after chunk 1, 13 after
            # chunk 3); ring loads for chunk g+1 are emitted after wave g
            # so no DMA queue head-of-line blocks on a future gather
            for g in range(G):
                fs = [3 * g, 3 * g + 1, 3 * g + 2]
                if g == 1:
                    fs.append(12)
                elif g == 3:
                    fs.append(13)
                for b in range(BL):
                    for nt in range(NA // 128):
                        sg(b, nt, fs, "act")
                if g + 1 < G:
                    emit_ring_loads(g + 1)

            ring_groups = [vis_loads, aud_loads1, aud_loads2, vt_loads]
            prev = None
            for grp in ring_groups:
                if not grp:
                    continue
                if prev is not None:
                    for h in grp:
                        add_dep_helper(h.ins, prev.ins, sync=False,
                                       reason="sync-ring class order")
                prev = grp[-1]

            # ---- mean over audio: ones-matmul accumulation on the PE ------
            pc128 = ps.tile([1, 128], F32, tag="pc", name="pc128", bufs=1)
            pcmm = [0]
            for g in range(G):
                for nt in range(NA // 128):
                    rhs = mx4[:, nt, :, g * 8:g * 8 + 8]
                    o = pc128[:].rearrange(
                        "p (b q) -> p b q", b=BL)[:, :, g * 8:g * 8 + 8]
                    mm = nc.tensor.matmul(
                        o, lhsT=ones[:], rhs=rhs,
                        start=(pcmm[0] == 0),
                        stop=(pcmm[0] == G * (NA // 128) - 1))
                    chain(prev_mm, mm, "pe chain")
                    pcmm[0] += 1

            # ---- tail: local softmax partials + one tiny AllReduce --------
            expm = wp.tile([1, 128], F32, tag="expm")
            nc.scalar.activation(expm[:], pc128[:], ACT.Exp, scale=SC)
            es = wp.tile([1, 4], F32, tag="es")
            nc.vector.tensor_reduce(
                out=es[:], in_=expm[:].rearrange("p (b q) -> p b q", b=4),
                axis=AX.X, op=ALU.add)
            lnes = wp.tile([1, 4], F32, tag="lnes")
            s1ln = wp.tile([1, 1], F32, tag="s1ln")
            nc.scalar.activation(lnes[:], es[:], ACT.Ln, accum_out=s1ln[:])
            dsc = wp.tile([1, 128], F32, tag="dsc")
            nc.vector.tensor_mul(dsc[:], pc128[:], dmask[:])
            d1 = wp.tile([1, 1], F32, tag="d1")
            nc.vector.reduce_sum(out=d1[:], in_=dsc[:], axis=AX.X)
            # E by local position via outer-product accumulation (puts E on
            # partitions), then map to global clip order: E_glob = E @ perm
            ecol = ps.tile([32, 1], F32, tag="pc", name="ecol", bufs=1)
            for b in range(BL):
                nc.tensor.matmul(ecol[:],
                                 lhsT=expm[0:1, b * 32:(b + 1) * 32],
                                 rhs=ones[0:1, 0:1],
                                 start=(b == 0), stop=(b == BL - 1))
            ecs = wp.tile([32, 1], F32, tag="ecs")
            nc.vector.tensor_copy(ecs[:], ecol[:])
            egl = ps.tile([1, 32], F32, tag="pc", name="egl", bufs=1)
            nc.tensor.matmul(egl[:], lhsT=ecs[:], rhs=perm[:],
                             start=True, stop=True)
            nc.vector.tensor_copy(vec[0:1, 0:32], egl[:])
            # w = (0.5*s1ln - d1) / B
            w0 = wp.tile([1, 1], F32, tag="w0")
            nc.vector.scalar_tensor_tensor(
                out=w0[:], in0=s1ln[:], scalar=0.5, in1=d1[:],
                op0=ALU.mult, op1=ALU.subtract)
            nc.scalar.mul(vec[0:1, 32:33], w0[:], 1.0 / B)

            ar_in = dp.tile([1, 64], F32, tag="ar_in", name="ar_in")
            nc.scalar.dma_start(out=ar_in[:], in_=vec[:])
            ar_out = dp.tile([1, 64], F32, tag="ar_out", name="ar_out",
                             addr_space="Shared")
            nc.gpsimd.collective_compute(
                "AllReduce", ALU.add, replica_groups=rg,
                ins=[ar_in[:, :].opt()], outs=[ar_out[:, :].opt()])

            rvec = wp.tile([1, 64], F32, tag="rvec")
            nc.sync.dma_start(out=rvec[:], in_=ar_out[:])
            lnE = wp.tile([1, 32], F32, tag="lnE")
            lnsum = wp.tile([1, 1], F32, tag="lnsum")
            nc.scalar.activation(lnE[:], rvec[0:1, 0:32], ACT.Ln,
                                 accum_out=lnsum[:])
            res = wp.tile([1, 1], F32, tag="res")
            nc.vector.scalar_tensor_tensor(
                out=res[:], in0=lnsum[:], scalar=0.5 / B,
                in1=rvec[0:1, 32:33], op0=ALU.mult, op1=ALU.add)
            nc.sync.dma_start(out=out[:], in_=res[:])

    nc.finalize()
    return nc


def _diag_mask(core):
    m = np.zeros((1, 128), dtype=np.float32)
    for b in range(BL):
        c = BL * core + b
        m[0, b * 32 + pos_of_clip(c, core)] = SC
    return m


def _perm(core):
    # E_glob[j] = sum_q E_loc[q] * PM[q, j]; j indexes global clip id
    m = np.zeros((32, 32), dtype=np.float32)
    for c in range(B):
        m[pos_of_clip(c, core), c] = 1.0
    return m


_NC_CACHE = None


def kernel(audio_feats: np.ndarray, visual_feats: np.ndarray) -> np.ndarray:
    from concourse.bass_utils import run_bass_kernel_spmd

    global _NC_CACHE
    if _NC_CACHE is None:
        _NC_CACHE = build()
    nc = _NC_CACHE

    audio = np.ascontiguousarray(audio_feats, dtype=np.float32)
    visual = np.ascontiguousarray(visual_feats, dtype=np.float32)
    in_maps = []
    for i in range(N_CORES):
        in_maps.append({
            "audio": audio[i * BL:(i + 1) * BL].reshape(AROWS, D),
            "visual": visual[i * BL:(i + 1) * BL].reshape(VROWS, D),
            "dmask": _diag_mask(i),
            "perm": _perm(i),
        })
    res = run_bass_kernel_spmd(nc, in_maps, core_ids=list(range(N_CORES)))
    val = res.results[0]["out"][0, 0]
    return np.asarray(val, dtype=np.float32)


if __name__ == "__main__":
    rng = np.random.default_rng(0)
    a = rng.standard_normal((B, NA, D)).astype(np.float32)
    v = rng.standard_normal((B, NV, D)).astype(np.float32)
    print(kernel(a, v))


# revision 18
# speedup vs baseline: 1.0202x; 1.0202x over previous
"""Distributed Trainium2 kernel for the audio-visual contrastive loss.

Math (reference):
    a = l2norm(audio)  (B=32, Na=512, D=768)
    v = l2norm(visual) (B=32, Nv=256, D=768)
    token_sims[b,c,n,m] = (a[b,n] . v[c,m]) / T
    clip_sims = mean_n max_m token_sims          (B, B)
    loss = mean_b -0.5*(log_softmax(clip)[b,b] + log_softmax(clip.T)[b,b])

Distribution over 8 NeuronCores (same collective plan as the proven
baseline): audio/visual batch sharded 4 clips/core, visual normalized,
fp8-cast, transposed d-major and AllGather'd in 4 chunks that pipeline
with the main loop; ring-relative consumption; scalar-loss AllReduce.

Main-loop engine plan (v4):
  - PE: k2-outer supergroups.  Each supergroup = one audio row-tile x
    the vTf blocks ready after the latest gather chunk; for each of
    the 3 DoubleRow contraction chunks one LDWEIGHTS feeds every block
    in the supergroup (a post-legalize pass drops the redundant
    LDWEIGHTS the legalizer inserts per matmul -- the PE array keeps
    its weights), cutting PE weight-load time ~4x.
  - All supergroups have an even number of blocks, and PSUM is three
    2-bank pair-tiles cycled round-robin: each pair drains with a
    single [128,1024] DVE max-reduce (the widest per-element-cheapest
    shape), right after its second bank's k2=2 matmul.  DVE is the
    only engine that can max-reduce PSUM, so everything else moves off
    it: prep copies go to ACT (except audio batch 1, on the first-MM
    critical path), row normalization of visual + audio batch 2 goes
    to GPSIMD normalize_recip, own-clip vTf copies go to GPSIMD.
  - mx columns are laid out in block-emission order (SEQ) so every
    pair-reduce writes one contiguous 4-column AP; the host-side
    dmask/perm inputs absorb the permutation.
  - The four AllGathers are not cross-gated on ring loads (separate
    buffers), so they run back-to-back on the CC engine and the last
    chunk lands ~40us earlier than with the baseline's gating.
"""

import sys

for _p in ("/opt/trn_rl_repo",):
    if _p not in sys.path:
        sys.path.insert(0, _p)

import numpy as np

import concourse.bacc as bacc
import concourse.bass as bass_mod
import concourse.mybir as mybir
import concourse.tile as tile
import concourse.tile_legalize as tl_mod
from concourse.tile_rust import add_dep_helper

N_CORES = 8
B = 32
NA = 512
NV = 256
D = 768
TEMPERATURE = 0.1
BL = B // N_CORES            # 4 clips per core
AROWS = BL * NA              # 2048 audio rows per core
VROWS = BL * NV              # 1024 visual rows per core
KD = D // 128                # 6 contraction chunks
KD2 = KD // 2                # 3 DoubleRow chunk-pairs
NT_A = AROWS // 128          # 16 audio row-tiles
NT_V = VROWS // 128          # 8 visual row-tiles
G = 4                        # visual AllGather chunks (1 clip/core each)
VCH = VROWS // G             # 256 visual rows per chunk per core
NPAIR = 3                    # PSUM pair-tiles (2 banks each) in the ring
USE_NR = True                # GPSIMD normalize_recip for vis + audio batch 2
F32 = mybir.dt.float32
BF16 = mybir.dt.bfloat16
FP8 = mybir.dt.float8e4
AX = mybir.AxisListType
ALU = mybir.AluOpType
ACT = mybir.ActivationFunctionType
SCL = 16.0                   # fp8 pre-scale (folded into the norm rsqrt)
SC = 1.0 / (NA * TEMPERATURE * SCL * SCL)   # psum-count -> clip_sims scale

# block emission order (wave structure); every wave is an even number of
# blocks so banks pair up cleanly
WAVE0 = [14, 15]
WAVES = [[0, 1], [2, 3, 4, 12], [5, 6, 7, 8], [9, 10, 11, 13]]
_EMIT = WAVE0 + [f for w in WAVES for f in w]


def _q_pair(f):
    # the two ring-relative positions (q = g*8 + t) covered by vTf block f,
    # in its column order
    if f >= 14:
        p = f - 14
        return (16 * p + 0, 16 * p + 8)
    if f >= 12:
        p = f - 12
        return (16 * p + 7, 16 * p + 15)
    g, u = divmod(f, 3)
    return (g * 8 + 2 * u + 1, g * 8 + 2 * u + 2)


# SEQ[q] = mx column (within a (nt,b) group) of ring-relative position q
SEQ = [0] * 32
for _i, _f in enumerate(_EMIT):
    _lo, _hi = _q_pair(_f)
    SEQ[_lo] = 2 * _i
    SEQ[_hi] = 2 * _i + 1
_SEQ_OF_BLOCK = {f: 2 * i for i, f in enumerate(_EMIT)}

# ---------------------------------------------------------------------------
# Post-legalize LDWEIGHTS dedupe.  tile_legalize splits every InstMatmult
# into InstLdweights + InstMatmult even when consecutive matmuls use
# identical stationary weights; a DoubleRow LDWEIGHTS costs ~156ns vs the
# ~107ns matmul stream, and the PE array keeps its weights between
# matmuls, so redundant reloads are pure loss.  Walk each block in final
# emission order tracking the loaded-weights signature and drop repeats,
# rewiring dep edges onto the matmul the dropped LDWEIGHTS was split from
# (which follows it in the same PE FIFO).  float32 weights are never
# deduped (f32 matmuls need their fused weight load).
# ---------------------------------------------------------------------------

_ORIG_LEGALIZE = tl_mod.tile_legalize


def _wsig(ldw):
    ap = ldw.ins[0]
    if "float32" in str(ap.dtype):
        return None
    bap = getattr(ap, "bass_ap", None)
    if bap is not None:
        key = (getattr(bap.tensor, "name", str(bap.tensor)), bap.offset,
               str(bap.ap))
    else:
        key = (getattr(ap, "memref", None), ap.offset, str(ap.ap))
    return (key, str(ap.dtype), str(ldw.perf_mode), bool(ldw.is_transpose),
            ldw.tile_position, ldw.tile_size)


def _dedup_tile_legalize(ordered, nc):
    out = _ORIG_LEGALIZE(ordered, nc)
    removed = []
    for bb, insts in out.items():
        cur = None
        keep = []
        n = len(insts)
        i = 0
        while i < n:
            inst = insts[i]
            if isinstance(inst, mybir.InstLdweights):
                sig = _wsig(inst)
                if sig is not None and cur is not None and sig == cur:
                    mm = None
                    for j in range(i + 1, n):
                        nxt = insts[j]
                        if isinstance(nxt, mybir.InstMatmult):
                            mm = nxt
                            break
                        if isinstance(nxt, mybir.InstLdweights):
                            break
                    if mm is not None:
                        removed.append((inst, mm))
                        i += 1
                        continue
                cur = sig
            keep.append(inst)
            i += 1
        out[bb] = keep
    if removed:
        rm_id = {id(l): m for l, m in removed}
        rm_nm = {l.name: m for l, m in removed}
        for bb, insts in out.items():
            for inst in insts:
                desc = getattr(inst, "descendants", None)
                if not desc:
                    continue
                new = []
                changed = False
                for d in desc:
                    m = rm_id.get(id(d)) or rm_nm.get(getattr(d, "name", None))
                    if m is not None:
                        new.append(m)
                        changed = True
                    else:
                        new.append(d)
                if changed:
                    inst.descendants = new
        for ldw, mm in removed:
            deps = getattr(ldw, "dependencies", None)
            if deps:
                cur = list(getattr(mm, "dependencies", None) or [])
                cur.extend(deps)
                mm.dependencies = cur
        imap = getattr(nc, "inst_map", None)
        if isinstance(imap, dict):
            for ldw, _ in removed:
                imap.pop(ldw.name, None)
    return out


def _install_dedup():
    if tl_mod.tile_legalize is not _dedup_tile_legalize:
        tl_mod.tile_legalize = _dedup_tile_legalize
    if tile.tile_legalize is not _dedup_tile_legalize:
        tile.tile_legalize = _dedup_tile_legalize


def pos_of_clip(c, core):
    # ring-relative positions: clip c = 4*r + g sits at q = g*8 + t where
    # t = (r - core) mod 8 is the ring step (t=0 is the core's own clip).
    g = c % G
    r = c // BL
    t = (r - core) % N_CORES
    return g * 8 + t


def seq_of_clip(c, core):
    return SEQ[pos_of_clip(c, core)]


def build():
    _install_dedup()
    nc = bacc.Bacc("TRN2", target_bir_lowering=False, debug=False,
                   num_devices=N_CORES)
    a_in = nc.declare_dram_parameter("audio", [AROWS, D], F32, isOutput=False)
    v_in = nc.declare_dram_parameter("visual", [VROWS, D], F32,
                                     isOutput=False)
    dmask_in = nc.declare_dram_parameter("dmask", [1, 128], F32,
                                         isOutput=False)
    perm_in = nc.declare_dram_parameter("perm", [32, 32], F32,
                                        isOutput=False)
    out = nc.declare_dram_parameter("out", [1, 1], F32, isOutput=True)
    ident_dram = nc.inline_tensor(np.eye(128, dtype=np.float32), name="ident")
    rg = [list(range(N_CORES))]

    with tile.TileContext(nc) as tc:
        with (
            tc.tile_pool(name="persist", bufs=1) as pp,
            tc.tile_pool(name="work", bufs=3) as wp,
            tc.tile_pool(name="ps", bufs=2, space="PSUM") as ps,
            tc.tile_pool(name="dram", bufs=1, space="DRAM") as dp,
        ):
            # ---- constants ------------------------------------------------
            ident_f32 = pp.tile([128, 128], F32, tag="identf")
            nc.sync.dma_start(out=ident_f32[:], in_=ident_dram[:])
            ident_bf = pp.tile([128, 128], BF16, tag="identb")
            nc.scalar.copy(ident_bf[:], ident_f32[:])
            ones = pp.tile([128, 1], F32, tag="ones")
            nc.gpsimd.memset(ones[:], 1.0)
            # warm the ACT Ln/Exp tables off the critical path
            wrma = wp.tile([1, 1], F32, tag="wrma")
            nc.vector.memset(wrma[:], 1.0)
            wrmb = wp.tile([1, 1], F32, tag="wrmb")
            nc.scalar.activation(wrmb[:], wrma[:], ACT.Exp)
            wrmc = wp.tile([1, 1], F32, tag="wrmc")
            nc.scalar.activation(wrmc[:], wrmb[:], ACT.Ln)
            vec = pp.tile([1, 64], F32, tag="vec")
            nc.vector.memset(vec[:], 0.0)

            # ---- persistent tensors ---------------------------------------
            VTW = N_CORES * VROWS        # 8192 vT columns per d-chunk
            aTf = [pp.tile([128, 2 * AROWS], FP8, tag=f"aT8{k2}",
                           name=f"aT8{k2}") for k2 in range(KD2)]
            vstall = pp.tile([128, KD * VROWS], FP8, tag="vstall")
            vTf = [pp.tile([128, 2 * VTW], FP8, tag=f"vT8{k2}",
                           name=f"vT8{k2}") for k2 in range(KD2)]
            mx = pp.tile([128, 512], F32, tag="mx")

            # ---- row-tile prep --------------------------------------------
            # one PSUM bank shared by all transposes: 8 rotating 256B
            # sub-slots (range-tracked deps).  Transposes run k-outer so
            # up to 4 consecutive tiles drain with one wide copy.
            ptbig = ps.tile([128, 1024], BF16, tag="pt", name="ptbig",
                            bufs=1)
            tr_ctr = [0]
            prev_pool = [None]

            def pool_chain(inst):
                if prev_pool[0] is not None:
                    add_dep_helper(inst.ins, prev_pool[0].ins, sync=False,
                                   reason="pool chain")
                prev_pool[0] = inst

            class BatchRec:
                pass

            def prep_batch(src, t0, nb, dst_of_k, load_group, use_nr,
                           cp_eng):
                rec = BatchRec()
                raws = []
                ssb = wp.tile([128, nb], F32, tag="ssb", name="ssb", bufs=2)
                for j in range(nb):
                    t = t0 + j
                    raw = wp.tile([128, D], F32, tag="raw", name="raw",
                                  bufs=10)
                    load_group.append(
                        nc.sync.dma_start(out=raw[:],
                                          in_=src[t * 128:(t + 1) * 128, :]))
                    sq = nc.scalar.activation(
                        wp.tile([128, D], F32, tag="sqs", name="sqs",
                                bufs=2)[:],
                        raw[:], ACT.Square, accum_out=ssb[:, j:j + 1])
                    if j == 0:
                        rec.sq_first = sq
                    raws.append(raw)
                nrm = wp.tile([128, nb], F32, tag="nrm", name="nrm", bufs=2)
                rec.cast_last = nc.scalar.activation(
                    nrm[:], ssb[:], ACT.Sqrt, scale=1.0 / (SCL * SCL))
                nbfs = []
                if use_nr:
                    # GPSIMD divides by the norm (and writes bf16) so the
                    # ACT engine stays free
                    for j in range(nb):
                        nbf = wp.tile([128, D], BF16, tag="nbf", name="nbf",
                                      bufs=10)
                        nr = nc.gpsimd.normalize_recip(
                            nbf[:], raws[j][:], nrm[:, j:j + 1])
                        pool_chain(nr)
                        nbfs.append(nbf)
                else:
                    rnb = wp.tile([128, nb], F32, tag="rnb", name="rnb",
                                  bufs=2)
                    nc.vector.reciprocal(rnb[:], nrm[:])
                    for j in range(nb):
                        nbf = wp.tile([128, D], BF16, tag="nbf", name="nbf",
                                      bufs=10)
                        rec.cast_last = nc.scalar.activation(
                            nbf[:], raws[j][:], ACT.Copy, bias=0.0,
                            scale=rnb[:, j:j + 1])
                        nbfs.append(nbf)
                rec.tr_first = rec.tr_last = None
                rec.cp_first = rec.cp_last = None
                for k in range(KD):
                    dst_tile, col = dst_of_k(t0, k)
                    c0 = 0
                    slot0 = tr_ctr[0] % 8
                    for j in range(nb):
                        slot = tr_ctr[0] % 8
                        tr_ctr[0] += 1
                        tr = nc.tensor.transpose(
                            ptbig[:, slot * 128:(slot + 1) * 128],
                            nbfs[j][:, 128 * k:128 * (k + 1)],
                            ident_bf[:])
                        if rec.tr_first is None:
                            rec.tr_first = tr
                        rec.tr_last = tr
                        if j + 1 == nb or (j + 1) % 4 == 0:
                            cp_fn = (cp_eng.tensor_copy
                                     if hasattr(cp_eng, "tensor_copy")
                                     else cp_eng.copy)
                            cp = cp_fn(
                                dst_tile[:, col + c0 * 128:
                                         col + (j + 1) * 128],
                                ptbig[:, slot0 * 128:
                                      (slot0 + (j + 1 - c0)) * 128])
                            if rec.cp_first is None:
                                rec.cp_first = cp
                            rec.cp_last = cp
                            c0 = j + 1
                            slot0 = tr_ctr[0] % 8
                return rec

            # ---- visual prep + bounce + chunked AllGather -----------------
            vis_loads, aud_loads1, aud_loads2 = [], [], []
            vt_loads = []
            vgath = []
            ag_insts = []
            vis_recs, aud_recs = [], []
            nbv = NT_V // G              # 2 tiles per chunk
            vst3 = vstall[:].rearrange("p (k c) -> p k c", k=KD)
            for g in range(G):
                vis_recs.append(prep_batch(
                    v_in, g * nbv, nbv,
                    lambda t0, k: (vstall, k * VROWS + (t0 // nbv) * VCH),
                    vis_loads, USE_NR, nc.vector))
                vb = dp.tile([128, KD * VCH // 4], F32, tag=f"vb{g}",
                             name=f"vb{g}")
                nc.scalar.dma_start(
                    out=vb[:, :].rearrange("p (k c) -> p k c", k=KD),
                    in_=vst3[:, :, g * VCH:(g + 1) * VCH].bitcast(F32))
                vg = dp.tile([N_CORES * 128, KD * VCH // 4], F32,
                             tag=f"vg{g}", name=f"vg{g}",
                             addr_space="Shared")
                cc = nc.gpsimd.collective_compute(
                    "AllGather", ALU.bypass, replica_groups=rg,
                    ins=[vb[:, :].opt()], outs=[vg[:, :].opt()])
                ag_insts.append(cc)
                vgath.append(vg)

            dmask = pp.tile([1, 128], F32, tag="dmask")
            vis_loads.append(nc.sync.dma_start(out=dmask[:], in_=dmask_in[:]))
            perm = pp.tile([32, 32], F32, tag="perm")
            vis_loads.append(nc.sync.dma_start(out=perm[:], in_=perm_in[:]))

            # ---- own clips: vstall -> vTf local blocks (GPSIMD, ----------
            # SBUF->SBUF; right after the visual normalize work) -----------
            loc_last = None
            for pair in range(2):
                fL = 14 + pair
                for k2 in range(KD2):
                    dst = vTf[k2][:, fL * 1024:(fL + 1) * 1024].bitcast(
                        F32).rearrange("p (ko h n) -> p ko h n", ko=2, h=2)
                    src = vst3[:, 2 * k2:2 * k2 + 2,
                               2 * pair * VCH:
                               (2 * pair + 2) * VCH].bitcast(F32).rearrange(
                        "p ko (h n) -> p ko h n", h=2)
                    cp = nc.gpsimd.tensor_copy(dst, src)
                    pool_chain(cp)
                    loc_last = cp

            # ---- audio prep (batch 1 via ACT casts + DVE copies for the
            # fastest lead-in; batch 2 via GPSIMD normalize + ACT copies) --
            aud_recs.append(prep_batch(
                a_in, 0, 8,
                lambda t0_, k: (aTf[k // 2], (k % 2) * AROWS + t0_ * 128),
                aud_loads1, False, nc.vector))
            aud_recs.append(prep_batch(
                a_in, 8, 8,
                lambda t0_, k: (aTf[k // 2], (k % 2) * AROWS + t0_ * 128),
                aud_loads2, USE_NR, nc.scalar))

            # ---- scheduler pins: visual prep, then audio prep, per engine
            add_dep_helper(aud_recs[0].sq_first.ins,
                           vis_recs[-1].cast_last.ins, sync=False,
                           reason="act: visual prep first")
            add_dep_helper(aud_recs[0].tr_first.ins,
                           vis_recs[-1].tr_last.ins, sync=False,
                           reason="pe: visual prep first")
            add_dep_helper(aud_recs[0].cp_first.ins,
                           vis_recs[-1].cp_last.ins, sync=False,
                           reason="dve: audio after visual")

            # ---- main loop ------------------------------------------------
            pairs = [ps.tile([128, 1024], F32, tag=f"pr{i}", name=f"pr{i}",
                             bufs=1) for i in range(NPAIR)]
            pair_reader = [None] * NPAIR
            pptr = [0]
            first_mm = [True]
            prev_mm = [None]
            prev_dve = [aud_recs[0].cp_last]
            prev_act = [aud_recs[-1].cast_last]
            mx4 = mx[:].rearrange("p (nt b q) -> p nt b q", nt=NA // 128,
                                  b=BL)

            def chain(prev_ref, inst, reason):
                if prev_ref[0] is not None:
                    add_dep_helper(inst.ins, prev_ref[0].ins, sync=False,
                                   reason=reason)
                prev_ref[0] = inst

            def sg(b, nt, fs):
                # fs has even length; consecutive block pairs share a
                # PSUM pair-tile and drain with one [128,1024] reduce
                lcol = (b * (NA // 128) + nt) * 128
                prs = []
                for _ in range(len(fs) // 2):
                    prs.append(pptr[0] % NPAIR)
                    pptr[0] += 1
                for k2 in range(KD2):
                    lhs3 = aTf[k2][:].rearrange(
                        "p (ko m) -> p ko m", ko=2)[:, :, lcol:lcol + 128]
                    for s, f in enumerate(fs):
                        rhs3 = vTf[k2][:].rearrange(
                            "p (f ko n) -> p f ko n", ko=2, n=512)[:, f]
                        pr = prs[s // 2]
                        half = s % 2
                        mm = nc.tensor.matmul(
                            pairs[pr][:, half * 512:(half + 1) * 512],
                            lhsT=lhs3, rhs=rhs3,
                            start=(k2 == 0), stop=(k2 == KD2 - 1),
                            perf_mode=mybir.MatmulPerfMode.DoubleRow)
                        if first_mm[0]:
                            add_dep_helper(mm.ins, aud_recs[0].tr_last.ins,
                                           sync=False,
                                           reason="pe prep before main")
                            first_mm[0] = False
                        chain(prev_mm, mm, "pe chain")
                        if k2 == 0 and pair_reader[pr] is not None:
                            add_dep_helper(mm.ins, pair_reader[pr].ins,
                                           sync=True, reason="pair WAR")
                for i in range(len(fs) // 2):
                    s0 = _SEQ_OF_BLOCK[fs[2 * i]]
                    rd = nc.vector.tensor_reduce(
                        out=mx4[:, nt, b, s0:s0 + 4],
                        in_=pairs[prs[i]][:].rearrange(
                            "p (c m) -> p c m", c=4),
                        axis=AX.X, op=ALU.max)
                    chain(prev_dve, rd, "dve chain")
                    pair_reader[prs[i]] = rd

            # ring loads for gather chunk g -> vTf (emitted per wave)
            pid = nc.sync.partition_id()
            pid_act = nc.scalar.partition_id()
            ROWBLK = 128 * (KD * VCH // 4)    # f32 elements per rank block

            def emit_ring_loads(g):
                for t in range(1, N_CORES):
                    if t == 7:
                        f, half = 12 + g // 2, g % 2
                    else:
                        f, half = 3 * g + (t - 1) // 2, (t - 1) % 2
                    r = (pid + t) % N_CORES
                    r_act = (pid_act + t) % N_CORES
                    blk0 = vgath[g][0:128, :].rearrange(
                        "p (k c) -> p k c", k=KD)
                    for k2 in range(KD2):
                        s_ap = blk0[:, 2 * k2:2 * k2 + 2, :]
                        roff = r if k2 < 2 else r_act
                        dyn = bass_mod.AP(
                            tensor=s_ap.tensor,
                            offset=roff * ROWBLK + s_ap.offset,
                            ap=s_ap.ap,
                            dep_tracking_offset=s_ap.offset)
                        dst = vTf[k2][:, f * 1024:(f + 1) * 1024].bitcast(
                            F32).rearrange("p (ko n) -> p ko n", ko=2)
                        if k2 < 2:
                            h1 = nc.sync.dma_start(
                                out=dst[:, :, half * 64:half * 64 + 64],
                                in_=dyn)
                            vt_loads.append(h1)
                        else:
                            h2 = nc.scalar.dma_start(
                                out=dst[:, :, half * 64:half * 64 + 64],
                                in_=dyn)
                            chain(prev_act, h2, "act chain")

            # wave 0: own clips (blocks 14,15) for all 16 audio tiles
            emit_ring_loads(0)
            for b in range(BL):
                for nt in range(NA // 128):
                    sg(b, nt, WAVE0)
            # chunk waves in gather-arrival order; ring loads for chunk
            # g+1 are emitted after wave g so no DMA queue head-of-line
            # blocks on a future gather
            for g, fs in enumerate(WAVES):
                for b in range(BL):
                    for nt in range(NA // 128):
                        sg(b, nt, fs)
                if g + 1 < G:
                    emit_ring_loads(g + 1)

            ring_groups = [vis_loads, aud_loads1, aud_loads2, vt_loads]
            prev = None
            for grp in ring_groups:
                if not grp:
                    continue
                if prev is not None:
                    for h in grp:
                        add_dep_helper(h.ins, prev.ins, sync=False,
                                       reason="sync-ring class order")
                prev = grp[-1]

            # ---- mean over audio: ones-matmul accumulation on the PE ------
            pc128 = ps.tile([1, 128], F32, tag="pc", name="pc128", bufs=1)
            pcmm = [0]
            for g in range(G):
                for nt in range(NA // 128):
                    rhs = mx4[:, nt, :, g * 8:g * 8 + 8]
                    o = pc128[:].rearrange(
                        "p (b q) -> p b q", b=BL)[:, :, g * 8:g * 8 + 8]
                    mm = nc.tensor.matmul(
                        o, lhsT=ones[:], rhs=rhs,
                        start=(pcmm[0] == 0),
                        stop=(pcmm[0] == G * (NA // 128) - 1))
                    chain(prev_mm, mm, "pe chain")
                    pcmm[0] += 1

            # ---- tail: local softmax partials + one tiny AllReduce --------
            expm = wp.tile([1, 128], F32, tag="expm")
            nc.scalar.activation(expm[:], pc128[:], ACT.Exp, scale=SC)
            es = wp.tile([1, 4], F32, tag="es")
            nc.vector.tensor_reduce(
                out=es[:], in_=expm[:].rearrange("p (b q) -> p b q", b=4),
                axis=AX.X, op=ALU.add)
            lnes = wp.tile([1, 4], F32, tag="lnes")
            s1ln = wp.tile([1, 1], F32, tag="s1ln")
            nc.scalar.activation(lnes[:], es[:], ACT.Ln, accum_out=s1ln[:])
            dsc = wp.tile([1, 128], F32, tag="dsc")
            nc.vector.tensor_mul(dsc[:], pc128[:], dmask[:])
            d1 = wp.tile([1, 1], F32, tag="d1")
            nc.vector.reduce_sum(out=d1[:], in_=dsc[:], axis=AX.X)
            # E by local position via outer-product accumulation (puts E on
            # partitions), then map to global clip order: E_glob = E @ perm
            ecol = ps.tile([32, 1], F32, tag="pc", name="ecol", bufs=1)
            for b in range(BL):
                nc.tensor.matmul(ecol[:],
                                 lhsT=expm[0:1, b * 32:(b + 1) * 32],
                                 rhs=ones[0:1, 0:1],
                                 start=(b == 0), stop=(b == BL - 1))
            ecs = wp.tile([32, 1], F32, tag="ecs")
            nc.vector.tensor_copy(ecs[:], ecol[:])
            egl = ps.tile([1, 32], F32, tag="pc", name="egl", bufs=1)
            nc.tensor.matmul(egl[:], lhsT=ecs[:], rhs=perm[:],
                             start=True, stop=True)
            nc.vector.tensor_copy(vec[0:1, 0:32], egl[:])
            # w = (0.5*s1ln - d1) / B
            w0 = wp.tile([1, 1], F32, tag="w0")
            nc.vector.scalar_tensor_tensor(
                out=w0[:], in0=s1ln[:], scalar=0.5, in1=d1[:],
                op0=ALU.mult, op1=ALU.subtract)
            nc.scalar.mul(vec[0:1, 32:33], w0[:], 1.0 / B)

            ar_in = dp.tile([1, 64], F32, tag="ar_in", name="ar_in")
            nc.scalar.dma_start(out=ar_in[:], in_=vec[:])
            ar_out = dp.tile([1, 64], F32, tag="ar_out", name="ar_out",
                             addr_space="Shared")
            nc.gpsimd.collective_compute(
                "AllReduce", ALU.add, replica_groups=rg,
                ins=[ar_in[:, :].opt()], outs=[ar_out[:, :].opt()])

            rvec = wp.tile([1, 64], F32, tag="rvec")
            nc.sync.dma_start(out=rvec[:], in_=ar_out[:])
            lnE = wp.tile([1, 32], F32, tag="lnE")
            lnsum = wp.tile([1, 1], F32, tag="lnsum")
            nc.scalar.activation(lnE[:], rvec[0:1, 0:32], ACT.Ln,
                                 accum_out=lnsum[:])
            res = wp.tile([1, 1], F32, tag="res")
            nc.vector.scalar_tensor_tensor(
                out=res[:], in0=lnsum[:], scalar=0.5 / B,
                in1=rvec[0:1, 32:33], op0=ALU.mult, op1=ALU.add)
            nc.sync.dma_start(out=out[:], in_=res[:])

    nc.finalize()
    return nc


def _diag_mask(core):
    m = np.zeros((1, 128), dtype=np.float32)
    for b in range(BL):
        c = BL * core + b
        m[0, b * 32 + seq_of_clip(c, core)] = SC
    return m


def _perm(core):
    # E_glob[j] = sum_s E_loc[s] * PM[s, j]; j indexes global clip id
    m = np.zeros((32, 32), dtype=np.float32)
    for c in range(B):
        m[seq_of_clip(c, core), c] = 1.0
    return m


_NC_CACHE = None


def kernel(audio_feats: np.ndarray, visual_feats: np.ndarray) -> np.ndarray:
    from concourse.bass_utils import run_bass_kernel_spmd

    global _NC_CACHE
    if _NC_CACHE is None:
        _NC_CACHE = build()
    nc = _NC_CACHE

    audio = np.ascontiguousarray(audio_feats, dtype=np.float32)
    visual = np.ascontiguousarray(visual_feats, dtype=np.float32)
    in_maps = []
    for i in range(N_CORES):
        in_maps.append({
            "audio": audio[i * BL:(i + 1) * BL].reshape(AROWS, D),
            "visual": visual[i * BL:(i + 1) * BL].reshape(VROWS, D),
            "dmask": _diag_mask(i),
            "perm": _perm(i),
        })
    res = run_bass_kernel_spmd(nc, in_maps, core_ids=list(range(N_CORES)))
    val = res.results[0]["out"][0, 0]
    return np.asarray(val, dtype=np.float32)


if __name__ == "__main__":
    rng = np.random.default_rng(0)
    a = rng.standard_normal((B, NA, D)).astype(np.float32)
    v = rng.standard_normal((B, NV, D)).astype(np.float32)
    print(kernel(a, v))
